# revision 14
# baseline (speedup 1.0000x reference)
"""CARAFE forward on 8 Trainium2 NeuronCores, data-parallel over batch.

Pixel-major reassembly design:
  - Host preps per sample: replicate-padded, transposed, kj-unrolled
    x as xun[66, 64, 3, 256] fp16 (row r, w, kj, c) plus channel-major
    xc[256, 4096] fp16 for the compressor.  Output is written in a
    device-friendly pixel-major layout and unscrambled on the host.
  - Compressor 1x1 conv (PE) -> sync-BN (AllReduce of sums) -> 3x3
    encoder conv with fused exp (PE+Act) -> softmax-over-h (DVE).
  - kern[36, 4096] is transposed per 128-pixel chunk on the PE to
    kT[128pix, 36] so reassembly products become per-partition-scalar
    multiplies (DVE tensor_scalar 4x mode / Act activation scale) --
    no partition-broadcast DMA at all.
  - Tap accumulation: identity-lhsT matmuls into PSUM (f32), one
    [128, 512] psum bank per (chunk, s-pair); straight Act copy to
    SBUF; contiguous DMA out.  A configurable subset of units instead
    accumulates fully on DVE in fp16 (scalar_tensor_tensor chains) to
    offload the PE; those go out through a separate fp16 tensor.
"""
import numpy as np

import concourse.bass as bass
import concourse.tile as tile
from concourse import bacc, mybir
from concourse.bass_utils import run_bass_kernel_spmd
from concourse.masks import make_identity

F32 = mybir.dt.float32
F16 = mybir.dt.float16
AX = mybir.AxisListType
OP = mybir.AluOpType
AF = mybir.ActivationFunctionType

B, C, H, W = 8, 256, 64, 64
CC = 64          # compressed channels
S = 2            # scale factor
K = 3            # kernel size
E = S * S * K * K  # 36 encoder channels
EPS = 1e-5
NCORES = 8
NPIX = H * W
NG = NPIX // 128          # 32 pixel chunks of 128 (2 rows each)
ROWB = W * K * C          # xun bytes-layout row: 64*3*256 elements
NCH = 8                   # h rows per conv chunk



def _ap(t, ap, extra_offset=0):
    return bass.AP(tensor=t.tensor, offset=t.offset + extra_offset, ap=ap)


def build():
    nc = bacc.Bacc("TRN2", target_bir_lowering=False, debug=False,
                   num_devices=NCORES)
    xun_d = nc.dram_tensor("xun", [H + 2, ROWB], F16, kind="ExternalInput").ap()
    xc_d = nc.dram_tensor("xc", [C, NPIX], F16, kind="ExternalInput").ap()
    w1_d = nc.dram_tensor("w1", [CC, C], F32, kind="ExternalInput").ap()
    xsum_d = nc.dram_tensor("xsum", [C, 1], F32, kind="ExternalInput").ap()
    xmom_d = nc.dram_tensor("xmom", [C, C], F32, kind="ExternalInput").ap()
    b1_d = nc.dram_tensor("b1", [CC, 1], F32, kind="ExternalInput").ap()
    gamma_d = nc.dram_tensor("gamma", [CC, 1], F32, kind="ExternalInput").ap()
    beta_d = nc.dram_tensor("beta", [CC, 1], F32, kind="ExternalInput").ap()
    w2_d = nc.dram_tensor("w2", [E, CC * K * K], F32, kind="ExternalInput").ap()
    b2_d = nc.dram_tensor("b2", [E, 1], F32, kind="ExternalInput").ap()
    o32_d = nc.dram_tensor("o32", [NG, 2, 128, 128 * S * S], F32,
                           kind="ExternalOutput").ap()

    with tile.TileContext(nc) as tc:
        with (
            tc.tile_pool(name="persist", bufs=1) as persist,
            tc.tile_pool(name="small", bufs=1) as small,
            tc.tile_pool(name="dram", bufs=1, space="DRAM") as dram,
        ):
            # ---------- constants & weights ----------
            ident = persist.tile([128, 128], F32)
            make_identity(nc, ident)
            ident16 = persist.tile([128, 128], F16)
            nc.vector.tensor_copy(out=ident16, in_=ident)

            with tc.tile_pool(name="warm", bufs=1, space="PSUM") as wps:
                wp = wps.tile([128, 128], F32, tag="w")
                for _ in range(25):
                    nc.tensor.matmul(wp, ident16, ident16, start=True,
                                     stop=True)

            w1_sb = small.tile([CC, C], F32)
            nc.sync.dma_start(out=w1_sb, in_=w1_d)
            w2_sb = small.tile([E, CC * K * K], F32)
            nc.sync.dma_start(out=w2_sb, in_=w2_d)
            b1_sb = small.tile([CC, 1], F32)
            nc.sync.dma_start(out=b1_sb, in_=b1_d)
            gamma_sb = small.tile([CC, 1], F32)
            nc.sync.dma_start(out=gamma_sb, in_=gamma_d)
            beta_sb = small.tile([CC, 1], F32)
            nc.sync.dma_start(out=beta_sb, in_=beta_d)
            b2_sb = small.tile([E, 1], F32)
            nc.sync.dma_start(out=b2_sb, in_=b2_d)

            # transposed weights via PE (stored fp16)
            w1T = persist.tile([128, 2, CC], F16)  # (c_part, chunk, o)
            w1T32 = persist.tile([128, 2, CC], F32)
            w2T = persist.tile([CC, K * K, E], F16)  # (c, tap, e)
            with tc.tile_pool(name="tp", bufs=2, space="PSUM") as tps:
                for ck in range(2):
                    pt = tps.tile([128, CC], F32, tag="w1t")
                    nc.tensor.transpose(pt, w1_sb[:, ck * 128:(ck + 1) * 128],
                                        ident[:CC, :CC])
                    nc.scalar.copy(out=w1T[:, ck, :], in_=pt)
                    nc.vector.tensor_copy(out=w1T32[:, ck, :], in_=pt)
                for t in range(K * K):
                    pt2 = tps.tile([CC, E], F32, tag="w2t")
                    src = _ap(w2_sb[:, :], [w2_sb[:, :].ap[0], [K * K, CC]],
                              extra_offset=t)
                    nc.tensor.transpose(pt2, src, ident[:E, :E])
                    nc.scalar.copy(out=w2T[:, t, :], in_=pt2)

            # ---------- channel-major x for the compressor ----------
            xc_sb = persist.tile([128, 2, NPIX], F16)
            for ck in range(2):
                nc.sync.dma_start(out=xc_sb[:, ck, :],
                                  in_=xc_d[ck * 128:(ck + 1) * 128, :])

            # ---------- BN affine from input sufficient statistics ----------
            # mu = w1 @ xsum / N + b1;  var = diag(w1 @ M @ w1.T)/N - (w1@xsum/N)^2
            xsum_sb = small.tile([128, 2], F32)
            for ck in range(2):
                nc.sync.dma_start(out=xsum_sb[:, ck:ck + 1],
                                  in_=xsum_d[ck * 128:(ck + 1) * 128, :])
            xmom_sb = small.tile([128, 2, C], F32)
            for ck in range(2):
                nc.sync.dma_start(out=xmom_sb[:, ck, :],
                                  in_=xmom_d[ck * 128:(ck + 1) * 128, :])
            m1 = small.tile([CC, 1], F32)
            q = small.tile([CC, 1], F32)
            qprod = small.tile([CC, C], F32)
            with tc.tile_pool(name="bnp", bufs=2, space="PSUM") as bnp:
                pT = bnp.tile([CC, C], F32, tag="wm")
                for ck in range(2):
                    nc.tensor.matmul(pT, w1T32[:, ck, :], xmom_sb[:, ck, :],
                                     start=(ck == 0), stop=(ck == 1))
                nc.vector.tensor_tensor(out=qprod, in0=pT, in1=w1_sb,
                                        op=OP.mult)
                nc.vector.tensor_reduce(out=q, in_=qprod, axis=AX.X, op=OP.add)
                pm = bnp.tile([CC, 1], F32, tag="ws")
                for ck in range(2):
                    nc.tensor.matmul(pm, w1T32[:, ck, :],
                                     xsum_sb[:, ck:ck + 1],
                                     start=(ck == 0), stop=(ck == 1))
                nc.vector.tensor_scalar_mul(out=m1, in0=pm,
                                            scalar1=1.0 / (B * NPIX))
            var = small.tile([CC, 1], F32)
            scl = small.tile([CC, 1], F32)
            bias2 = small.tile([CC, 1], F32)
            eps_sb = small.tile([CC, 1], F32)
            nc.vector.memset(eps_sb, EPS)
            nc.vector.tensor_scalar_mul(out=var, in0=q,
                                        scalar1=1.0 / (B * NPIX))
            nc.vector.tensor_tensor(out=scl, in0=m1, in1=m1, op=OP.mult)
            nc.vector.tensor_tensor(out=var, in0=var, in1=scl, op=OP.subtract)
            nc.scalar.activation(out=var, in_=var, func=AF.Sqrt, bias=eps_sb,
                                 scale=1.0)
            nc.vector.reciprocal(out=var, in_=var)
            nc.vector.tensor_tensor(out=scl, in0=gamma_sb, in1=var,
                                    op=OP.mult)
            nc.vector.tensor_tensor(out=bias2, in0=m1, in1=scl, op=OP.mult)
            nc.vector.tensor_tensor(out=bias2, in0=beta_sb, in1=bias2,
                                    op=OP.subtract)

            # ---------- compressor with fused BN + ReLU (zero-padded) ----------
            comp = persist.tile([CC, H + 2, W + 2], F16)
            nc.vector.memset(comp[:, 0:1, :], 0.0)
            nc.vector.memset(comp[:, H + 1:H + 2, :], 0.0)
            nc.vector.memset(comp[:, :, 0:1], 0.0)
            nc.vector.memset(comp[:, :, W + 1:W + 2], 0.0)
            with tc.tile_pool(name="cps", bufs=2, space="PSUM") as cps:
                for hc in range(H // NCH):
                    pc = cps.tile([CC, NCH, W], F32, tag="comp")
                    for ck in range(2):
                        nc.tensor.matmul(
                            pc, w1T[:, ck, :],
                            _ap(xc_sb[:, ck, :],
                                [xc_sb[:, ck, :].ap[0], [W, NCH], [1, W]],
                                extra_offset=hc * NCH * W),
                            start=(ck == 0), stop=(ck == 1))
                    nc.scalar.activation(
                        out=comp[:, 1 + hc * NCH:1 + (hc + 1) * NCH, 1:W + 1],
                        in_=pc, func=AF.Relu, bias=bias2, scale=scl)

            # ---------- encoder conv + fused exp ----------
            eexp = persist.tile([E, H, W], F16)
            with tc.tile_pool(name="eps", bufs=2, space="PSUM") as eps_pool:
                for hc in range(H // NCH):
                    pe = eps_pool.tile([E, NCH, W], F32, tag="enc")
                    for t in range(K * K):
                        ki, kj = t // K, t % K
                        nc.tensor.matmul(
                            pe, w2T[:, t, :],
                            comp[:, hc * NCH + ki:hc * NCH + ki + NCH,
                                 kj:kj + W],
                            start=(t == 0), stop=(t == K * K - 1))
                    nc.scalar.activation(
                        out=eexp[:, hc * NCH:(hc + 1) * NCH, :], in_=pe,
                        func=AF.Exp, bias=b2_sb, scale=1.0)

            # ---------- softmax over h (partial sums per conv chunk) ----------
            zpart = small.tile([E, H // NCH, W], F32)
            for hc in range(H // NCH):
                eec = eexp[:, hc * NCH:(hc + 1) * NCH, :]
                ee_wh = _ap(eec, [eec.ap[0], [1, W], [W, NCH]])
                nc.vector.tensor_reduce(out=zpart[:, hc, :], in_=ee_wh,
                                        axis=AX.X, op=OP.add)
            zrec = small.tile([E, W], F32)
            zp = zpart[:, :, :]
            nc.vector.tensor_reduce(out=zrec,
                                    in_=_ap(zp, [zp.ap[0], [1, W],
                                                 [W, H // NCH]]),
                                    axis=AX.X, op=OP.add)
            nc.vector.reciprocal(out=zrec, in_=zrec)
            zrec16 = small.tile([E, W], F16)
            nc.vector.tensor_copy(out=zrec16, in_=zrec)
            kern = persist.tile([E, H, W], F16)
            zb = zrec16[:, :]
            nc.vector.tensor_tensor(
                out=kern, in0=eexp[:, :, :],
                in1=_ap(zb, [zb.ap[0], [0, H], [1, W]]),
                op=OP.mult)

            # ---------- kern -> pixel-major kT via PE transposes ----------
            kT = persist.tile([128, NG, E], F32)
            kern_f = kern.rearrange("p a b -> p (a b)")
            with tc.tile_pool(name="ktp", bufs=4, space="PSUM") as ktp:
                for g in range(NG):
                    pk = ktp.tile([128, E], F16, tag="kt")
                    nc.tensor.transpose(pk, kern_f[:, g * 128:(g + 1) * 128],
                                        ident16[:E, :E])
                    nc.vector.tensor_copy(out=kT[:, g, :], in_=pk)

            # ---------- reassembly ----------
            # psum[c, (s, pix)] += sum_t xu_t[pix', c].T @ D4_t[pix', (s, pix)]
            # with D4_t[p, (s, pix)] = kern[s*9+t, pix] * delta(p, pix):
            # the PE applies the per-pixel kernel weights and accumulates
            # the 9 taps in one pass; DVE/Act only build the diagonals.
            with (
                tc.tile_pool(name="xu", bufs=12) as xup,
                tc.tile_pool(name="d4", bufs=18) as d4p,
                tc.tile_pool(name="stage", bufs=6) as stagep,
                tc.tile_pool(name="mps", bufs=4, space="PSUM") as mps,
            ):
                for g in range(NG):
                    xu = xup.tile([128, K, K * C], F16)
                    for hh in range(2):
                        src = _ap(xun_d, [[ROWB // W, W], [ROWB, K],
                                          [1, K * C]],
                                  extra_offset=(2 * g + hh) * ROWB)
                        nc.sync.dma_start(
                            out=xu[hh * 64:(hh + 1) * 64, :, :], in_=src)
                    pss = [mps.tile([128, S * S, 128], F32, tag=f"acc{ck}",
                                    name=f"ps_{g}_{ck}") for ck in range(2)]
                    for t in range(K * K):
                        ki, kj = t // K, t % K
                        d4 = d4p.tile([128, S * S, 128], F16)
                        nc.vector.tensor_tensor(
                            out=d4[:, 0:3, :],
                            in0=_ap(ident16, [ident16[:, :].ap[0], [0, 3],
                                              [1, 128]]),
                            in1=_ap(kT[:, g, :], [kT[:, g, :].ap[0],
                                                  [K * K, 3], [0, 128]],
                                    extra_offset=t),
                            op=OP.mult)
                        nc.scalar.activation(
                            out=d4[:, 3, :], in_=ident16, func=AF.Identity,
                            scale=kT[:, g, 3 * K * K + t:3 * K * K + t + 1])
                        d4f = d4.rearrange("p a b -> p (a b)")
                        for ck in range(2):
                            nc.tensor.matmul(
                                pss[ck],
                                xu[:, ki, kj * C + ck * 128:
                                   kj * C + ck * 128 + 128],
                                d4f, start=(t == 0), stop=(t == K * K - 1))
                    for ck in range(2):
                        stg = stagep.tile([128, S * S * 128], F32)
                        if ck == 0 or g % 2 == 0:
                            nc.scalar.copy(out=stg, in_=pss[ck])
                        else:
                            nc.vector.tensor_copy(out=stg, in_=pss[ck])
                        dst = _ap(o32_d, [[S * S * 128, 128], [1, S * S * 128]],
                                  extra_offset=(g * 2 + ck) * 128 * S * S * 128)
                        nc.sync.dma_start(out=dst, in_=stg)
    nc.compile()
    return nc


_NC_CACHE = None


def _get_nc():
    global _NC_CACHE
    if _NC_CACHE is None:
        _NC_CACHE = build()
    return _NC_CACHE


def _make_in_maps(inputs):
    x = np.ascontiguousarray(np.asarray(inputs["x"], dtype=np.float32))
    w1 = np.ascontiguousarray(np.asarray(inputs["w1"], dtype=np.float32))
    b1 = np.asarray(inputs["b1"], dtype=np.float32).reshape(CC, 1)
    gamma = np.asarray(inputs["gamma"], dtype=np.float32).reshape(CC, 1)
    beta = np.asarray(inputs["beta"], dtype=np.float32).reshape(CC, 1)
    w2 = np.asarray(inputs["w2"], dtype=np.float32).reshape(E, CC * K * K)
    b2 = np.asarray(inputs["b2"], dtype=np.float32).reshape(E, 1)
    xflat = x.transpose(1, 0, 2, 3).reshape(C, B * NPIX).astype(np.float64)
    xsum = np.ascontiguousarray(
        xflat.sum(axis=1, keepdims=True).astype(np.float32))
    xmom = np.ascontiguousarray(
        (xflat @ xflat.T).astype(np.float32))
    in_maps = []
    for b in range(NCORES):
        xb = x[b]                                   # (256, 64, 64)
        xp = np.pad(xb, ((0, 0), (1, 1), (1, 1)), mode="edge")
        xt = xp.transpose(1, 2, 0)                  # (66, 66, 256)
        xun = np.stack([xt[:, kj:kj + W, :] for kj in range(K)], axis=2)
        in_maps.append({
            "xsum": xsum,
            "xmom": xmom,
            "xun": np.ascontiguousarray(
                xun.reshape(H + 2, ROWB).astype(np.float16)),
            "xc": np.ascontiguousarray(
                xb.reshape(C, NPIX).astype(np.float16)),
            "w1": w1,
            "b1": np.ascontiguousarray(b1),
            "gamma": np.ascontiguousarray(gamma),
            "beta": np.ascontiguousarray(beta),
            "w2": np.ascontiguousarray(w2),
            "b2": np.ascontiguousarray(b2),
        })
    return in_maps


def _unscramble(o32):
    """Device (NG, 2ck, 128c, 4s*128pix) f32 -> reference (256, 128, 128)."""
    v = np.asarray(o32, dtype=np.float32).reshape(NG, 2, 128, S * S, 128)
    arr = v.transpose(3, 1, 2, 0, 4).reshape(S * S, C, NPIX)  # s, c, n
    a2 = arr.reshape(S * S, C // 4, 2, 2, H, W)  # s, c4, hb, wb, h, w
    return np.ascontiguousarray(
        a2.transpose(0, 1, 4, 2, 3, 5)).reshape(C, S * H, S * W)


def kernel(x, w1, b1, gamma, beta, w2, b2, **kwargs):
    in_maps = _make_in_maps(dict(x=x, w1=w1, b1=b1, gamma=gamma, beta=beta,
                                 w2=w2, b2=b2))
    nc = _get_nc()
    res = run_bass_kernel_spmd(nc, in_maps, core_ids=list(range(NCORES)))
    return np.stack([_unscramble(res.results[b]["o32"])
                     for b in range(NCORES)], axis=0)


# revision 15
# speedup vs baseline: 1.1186x; 1.1186x over previous
"""CARAFE forward on 8 Trainium2 NeuronCores, data-parallel over batch.

Pixel-major reassembly design:
  - Host preps per sample: replicate-padded, transposed, kj-unrolled
    x as xun[66, 64, 3, 256] fp16 (row r, w, kj, c) plus channel-major
    xc[256, 4096] fp16 for the compressor.  Output is written in a
    device-friendly pixel-major layout and unscrambled on the host.
  - Compressor 1x1 conv (PE) -> sync-BN (AllReduce of sums) -> 3x3
    encoder conv with fused exp (PE+Act) -> softmax-over-h (DVE).
  - kern[36, 4096] is transposed per 128-pixel chunk on the PE to
    kT[128pix, 36] so reassembly products become per-partition-scalar
    multiplies (DVE tensor_scalar 4x mode / Act activation scale) --
    no partition-broadcast DMA at all.
  - Tap accumulation: identity-lhsT matmuls into PSUM (f32), one
    [128, 512] psum bank per (chunk, s-pair); straight Act copy to
    SBUF; contiguous DMA out.  A configurable subset of units instead
    accumulates fully on DVE in fp16 (scalar_tensor_tensor chains) to
    offload the PE; those go out through a separate fp16 tensor.
"""
import numpy as np

import concourse.bass as bass
import concourse.tile as tile
from concourse import bacc, mybir
from concourse.bass_utils import run_bass_kernel_spmd
from concourse.masks import make_identity

F32 = mybir.dt.float32
F16 = mybir.dt.float16
AX = mybir.AxisListType
OP = mybir.AluOpType
AF = mybir.ActivationFunctionType

B, C, H, W = 8, 256, 64, 64
CC = 64          # compressed channels
S = 2            # scale factor
K = 3            # kernel size
E = S * S * K * K  # 36 encoder channels
EPS = 1e-5
NCORES = 8
NPIX = H * W
NG = NPIX // 128          # 32 pixel chunks of 128 (2 rows each)
ROWB = W * K * C          # xun bytes-layout row: 64*3*256 elements
NCH = 8                   # h rows per conv chunk



def _ap(t, ap, extra_offset=0):
    return bass.AP(tensor=t.tensor, offset=t.offset + extra_offset, ap=ap)


def build():
    nc = bacc.Bacc("TRN2", target_bir_lowering=False, debug=False,
                   num_devices=NCORES)
    xun_d = nc.dram_tensor("xun", [H + 2, ROWB], F16, kind="ExternalInput").ap()
    xc_d = nc.dram_tensor("xc", [C, NPIX], F16, kind="ExternalInput").ap()
    w1_d = nc.dram_tensor("w1", [CC, C], F32, kind="ExternalInput").ap()
    xsum_d = nc.dram_tensor("xsum", [C, 1], F32, kind="ExternalInput").ap()
    xmom_d = nc.dram_tensor("xmom", [C, C], F32, kind="ExternalInput").ap()
    b1_d = nc.dram_tensor("b1", [CC, 1], F32, kind="ExternalInput").ap()
    gamma_d = nc.dram_tensor("gamma", [CC, 1], F32, kind="ExternalInput").ap()
    beta_d = nc.dram_tensor("beta", [CC, 1], F32, kind="ExternalInput").ap()
    w2_d = nc.dram_tensor("w2", [E, CC * K * K], F32, kind="ExternalInput").ap()
    b2_d = nc.dram_tensor("b2", [E, 1], F32, kind="ExternalInput").ap()
    o32_d = nc.dram_tensor("o32", [NG, 2, 128, 128 * S * S], F32,
                           kind="ExternalOutput").ap()

    with tile.TileContext(nc) as tc:
        with (
            tc.tile_pool(name="persist", bufs=1) as persist,
            tc.tile_pool(name="small", bufs=1) as small,
            tc.tile_pool(name="dram", bufs=1, space="DRAM") as dram,
        ):
            # ---------- constants & weights ----------
            ident = persist.tile([128, 128], F32)
            make_identity(nc, ident)
            ident16 = persist.tile([128, 128], F16)
            nc.vector.tensor_copy(out=ident16, in_=ident)

            w1_sb = small.tile([CC, C], F32)
            nc.sync.dma_start(out=w1_sb, in_=w1_d)
            w2_sb = small.tile([E, CC * K * K], F32)
            nc.sync.dma_start(out=w2_sb, in_=w2_d)
            b1_sb = small.tile([CC, 1], F32)
            nc.sync.dma_start(out=b1_sb, in_=b1_d)
            gamma_sb = small.tile([CC, 1], F32)
            nc.sync.dma_start(out=gamma_sb, in_=gamma_d)
            beta_sb = small.tile([CC, 1], F32)
            nc.sync.dma_start(out=beta_sb, in_=beta_d)
            b2_sb = small.tile([E, 1], F32)
            nc.sync.dma_start(out=b2_sb, in_=b2_d)

            # transposed weights via PE (stored fp16)
            w1T = persist.tile([128, 2, CC], F16)  # (c_part, chunk, o)
            w1T32 = persist.tile([128, 2, CC], F32)
            w2T = persist.tile([CC, K * K, E], F16)  # (c, tap, e)
            with tc.tile_pool(name="tp", bufs=2, space="PSUM") as tps:
                for ck in range(2):
                    pt = tps.tile([128, CC], F32, tag="w1t")
                    nc.tensor.transpose(pt, w1_sb[:, ck * 128:(ck + 1) * 128],
                                        ident[:CC, :CC])
                    nc.scalar.copy(out=w1T[:, ck, :], in_=pt)
                    nc.vector.tensor_copy(out=w1T32[:, ck, :], in_=pt)
                for t in range(K * K):
                    pt2 = tps.tile([CC, E], F32, tag="w2t")
                    src = _ap(w2_sb[:, :], [w2_sb[:, :].ap[0], [K * K, CC]],
                              extra_offset=t)
                    nc.tensor.transpose(pt2, src, ident[:E, :E])
                    nc.scalar.copy(out=w2T[:, t, :], in_=pt2)

            # ---------- channel-major x for the compressor ----------
            xc_sb = persist.tile([128, 2, NPIX], F16)
            for ck in range(2):
                nc.sync.dma_start(out=xc_sb[:, ck, :],
                                  in_=xc_d[ck * 128:(ck + 1) * 128, :])

            # ---------- BN affine from input sufficient statistics ----------
            # mu = w1 @ xsum / N + b1;  var = diag(w1 @ M @ w1.T)/N - (w1@xsum/N)^2
            xsum_sb = small.tile([128, 2], F32)
            for ck in range(2):
                nc.sync.dma_start(out=xsum_sb[:, ck:ck + 1],
                                  in_=xsum_d[ck * 128:(ck + 1) * 128, :])
            xmom_sb = small.tile([128, 2, C], F32)
            for ck in range(2):
                nc.sync.dma_start(out=xmom_sb[:, ck, :],
                                  in_=xmom_d[ck * 128:(ck + 1) * 128, :])
            m1 = small.tile([CC, 1], F32)
            q = small.tile([CC, 1], F32)
            qprod = small.tile([CC, C], F32)
            with tc.tile_pool(name="bnp", bufs=2, space="PSUM") as bnp:
                pT = bnp.tile([CC, C], F32, tag="wm")
                for ck in range(2):
                    nc.tensor.matmul(pT, w1T32[:, ck, :], xmom_sb[:, ck, :],
                                     start=(ck == 0), stop=(ck == 1))
                nc.vector.tensor_tensor(out=qprod, in0=pT, in1=w1_sb,
                                        op=OP.mult)
                nc.vector.tensor_reduce(out=q, in_=qprod, axis=AX.X, op=OP.add)
                pm = bnp.tile([CC, 1], F32, tag="ws")
                for ck in range(2):
                    nc.tensor.matmul(pm, w1T32[:, ck, :],
                                     xsum_sb[:, ck:ck + 1],
                                     start=(ck == 0), stop=(ck == 1))
                nc.vector.tensor_scalar_mul(out=m1, in0=pm,
                                            scalar1=1.0 / (B * NPIX))
            var = small.tile([CC, 1], F32)
            scl = small.tile([CC, 1], F32)
            bias2 = small.tile([CC, 1], F32)
            eps_sb = small.tile([CC, 1], F32)
            nc.vector.memset(eps_sb, EPS)
            nc.vector.tensor_scalar_mul(out=var, in0=q,
                                        scalar1=1.0 / (B * NPIX))
            nc.vector.tensor_tensor(out=scl, in0=m1, in1=m1, op=OP.mult)
            nc.vector.tensor_tensor(out=var, in0=var, in1=scl, op=OP.subtract)
            nc.scalar.activation(out=var, in_=var, func=AF.Sqrt, bias=eps_sb,
                                 scale=1.0)
            nc.vector.reciprocal(out=var, in_=var)
            nc.vector.tensor_tensor(out=scl, in0=gamma_sb, in1=var,
                                    op=OP.mult)
            nc.vector.tensor_tensor(out=bias2, in0=m1, in1=scl, op=OP.mult)
            nc.vector.tensor_tensor(out=bias2, in0=beta_sb, in1=bias2,
                                    op=OP.subtract)

            # ---------- compressor with fused BN + ReLU (zero-padded) ----------
            comp = persist.tile([CC, H + 2, W + 2], F16)
            nc.vector.memset(comp[:, 0:1, :], 0.0)
            nc.vector.memset(comp[:, H + 1:H + 2, :], 0.0)
            nc.vector.memset(comp[:, :, 0:1], 0.0)
            nc.vector.memset(comp[:, :, W + 1:W + 2], 0.0)
            with tc.tile_pool(name="cps", bufs=2, space="PSUM") as cps:
                for hc in range(H // NCH):
                    pc = cps.tile([CC, NCH, W], F32, tag="comp")
                    for ck in range(2):
                        nc.tensor.matmul(
                            pc, w1T[:, ck, :],
                            _ap(xc_sb[:, ck, :],
                                [xc_sb[:, ck, :].ap[0], [W, NCH], [1, W]],
                                extra_offset=hc * NCH * W),
                            start=(ck == 0), stop=(ck == 1))
                    nc.scalar.activation(
                        out=comp[:, 1 + hc * NCH:1 + (hc + 1) * NCH, 1:W + 1],
                        in_=pc, func=AF.Relu, bias=bias2, scale=scl)

            # ---------- encoder conv + fused exp ----------
            eexp = persist.tile([E, H, W], F16)
            with tc.tile_pool(name="eps", bufs=2, space="PSUM") as eps_pool:
                for hc in range(H // NCH):
                    pe = eps_pool.tile([E, NCH, W], F32, tag="enc")
                    for t in range(K * K):
                        ki, kj = t // K, t % K
                        nc.tensor.matmul(
                            pe, w2T[:, t, :],
                            comp[:, hc * NCH + ki:hc * NCH + ki + NCH,
                                 kj:kj + W],
                            start=(t == 0), stop=(t == K * K - 1))
                    nc.scalar.activation(
                        out=eexp[:, hc * NCH:(hc + 1) * NCH, :], in_=pe,
                        func=AF.Exp, bias=b2_sb, scale=1.0)

            # ---------- softmax over h (partial sums per conv chunk) ----------
            zpart = small.tile([E, H // NCH, W], F32)
            for hc in range(H // NCH):
                eec = eexp[:, hc * NCH:(hc + 1) * NCH, :]
                ee_wh = _ap(eec, [eec.ap[0], [1, W], [W, NCH]])
                nc.vector.tensor_reduce(out=zpart[:, hc, :], in_=ee_wh,
                                        axis=AX.X, op=OP.add)
            zrec = small.tile([E, W], F32)
            zp = zpart[:, :, :]
            nc.vector.tensor_reduce(out=zrec,
                                    in_=_ap(zp, [zp.ap[0], [1, W],
                                                 [W, H // NCH]]),
                                    axis=AX.X, op=OP.add)
            nc.vector.reciprocal(out=zrec, in_=zrec)
            zrec16 = small.tile([E, W], F16)
            nc.vector.tensor_copy(out=zrec16, in_=zrec)
            kern = persist.tile([E, H, W], F16)
            zb = zrec16[:, :]
            nc.vector.tensor_tensor(
                out=kern, in0=eexp[:, :, :],
                in1=_ap(zb, [zb.ap[0], [0, H], [1, W]]),
                op=OP.mult)

            # ---------- kern -> pixel-major kT via PE transposes ----------
            kT = persist.tile([128, NG, E], F32)
            kern_f = kern.rearrange("p a b -> p (a b)")
            with tc.tile_pool(name="ktp", bufs=4, space="PSUM") as ktp:
                for g in range(NG):
                    pk = ktp.tile([128, E], F16, tag="kt")
                    nc.tensor.transpose(pk, kern_f[:, g * 128:(g + 1) * 128],
                                        ident16[:E, :E])
                    nc.vector.tensor_copy(out=kT[:, g, :], in_=pk)

            # ---------- reassembly ----------
            # psum[c, (s, pix)] += sum_t xu_t[pix', c].T @ D4_t[pix', (s, pix)]
            # with D4_t[p, (s, pix)] = kern[s*9+t, pix] * delta(p, pix):
            # the PE applies the per-pixel kernel weights and accumulates
            # the 9 taps in one pass; DVE/Act only build the diagonals.
            with (
                tc.tile_pool(name="xu", bufs=12) as xup,
                tc.tile_pool(name="d4", bufs=10) as d4p,
                tc.tile_pool(name="stage", bufs=6) as stagep,
                tc.tile_pool(name="mps", bufs=4, space="PSUM") as mps,
            ):
                for g in range(NG):
                    xu = xup.tile([128, K, K * C], F16)
                    for hh in range(2):
                        src = _ap(xun_d, [[ROWB // W, W], [ROWB, K],
                                          [1, K * C]],
                                  extra_offset=(2 * g + hh) * ROWB)
                        nc.sync.dma_start(
                            out=xu[hh * 64:(hh + 1) * 64, :, :], in_=src)
                    pss = [mps.tile([128, S * S, 128], F32, tag=f"acc{ck}",
                                    name=f"ps_{g}_{ck}") for ck in range(2)]
                    for t in range(K * K):
                        ki, kj = t // K, t % K
                        d4 = d4p.tile([128, S * S, 128], F16)
                        nc.vector.tensor_tensor(
                            out=d4[:, 0:3, :],
                            in0=_ap(ident16, [ident16[:, :].ap[0], [0, 3],
                                              [1, 128]]),
                            in1=_ap(kT[:, g, :], [kT[:, g, :].ap[0],
                                                  [K * K, 3], [0, 128]],
                                    extra_offset=t),
                            op=OP.mult)
                        nc.scalar.activation(
                            out=d4[:, 3, :], in_=ident16, func=AF.Identity,
                            scale=kT[:, g, 3 * K * K + t:3 * K * K + t + 1])
                        d4f = d4.rearrange("p a b -> p (a b)")
                        for ck in range(2):
                            nc.tensor.matmul(
                                pss[ck],
                                xu[:, ki, kj * C + ck * 128:
                                   kj * C + ck * 128 + 128],
                                d4f, start=(t == 0), stop=(t == K * K - 1))
                    for ck in range(2):
                        stg = stagep.tile([128, S * S * 128], F32)
                        if ck == 0:
                            nc.scalar.copy(out=stg, in_=pss[ck])
                        else:
                            nc.vector.tensor_copy(out=stg, in_=pss[ck])
                        dst = _ap(o32_d, [[S * S * 128, 128], [1, S * S * 128]],
                                  extra_offset=(g * 2 + ck) * 128 * S * S * 128)
                        nc.sync.dma_start(out=dst, in_=stg)
    nc.compile()
    return nc


_NC_CACHE = None


def _get_nc():
    global _NC_CACHE
    if _NC_CACHE is None:
        _NC_CACHE = build()
    return _NC_CACHE


def _make_in_maps(inputs):
    x = np.ascontiguousarray(np.asarray(inputs["x"], dtype=np.float32))
    w1 = np.ascontiguousarray(np.asarray(inputs["w1"], dtype=np.float32))
    b1 = np.asarray(inputs["b1"], dtype=np.float32).reshape(CC, 1)
    gamma = np.asarray(inputs["gamma"], dtype=np.float32).reshape(CC, 1)
    beta = np.asarray(inputs["beta"], dtype=np.float32).reshape(CC, 1)
    w2 = np.asarray(inputs["w2"], dtype=np.float32).reshape(E, CC * K * K)
    b2 = np.asarray(inputs["b2"], dtype=np.float32).reshape(E, 1)
    xflat = x.transpose(1, 0, 2, 3).reshape(C, B * NPIX).astype(np.float64)
    xsum = np.ascontiguousarray(
        xflat.sum(axis=1, keepdims=True).astype(np.float32))
    xmom = np.ascontiguousarray(
        (xflat @ xflat.T).astype(np.float32))
    in_maps = []
    for b in range(NCORES):
        xb = x[b]                                   # (256, 64, 64)
        xp = np.pad(xb, ((0, 0), (1, 1), (1, 1)), mode="edge")
        xt = xp.transpose(1, 2, 0)                  # (66, 66, 256)
        xun = np.stack([xt[:, kj:kj + W, :] for kj in range(K)], axis=2)
        in_maps.append({
            "xsum": xsum,
            "xmom": xmom,
            "xun": np.ascontiguousarray(
                xun.reshape(H + 2, ROWB).astype(np.float16)),
            "xc": np.ascontiguousarray(
                xb.reshape(C, NPIX).astype(np.float16)),
            "w1": w1,
            "b1": np.ascontiguousarray(b1),
            "gamma": np.ascontiguousarray(gamma),
            "beta": np.ascontiguousarray(beta),
            "w2": np.ascontiguousarray(w2),
            "b2": np.ascontiguousarray(b2),
        })
    return in_maps


def _unscramble(o32):
    """Device (NG, 2ck, 128c, 4s*128pix) f32 -> reference (256, 128, 128)."""
    v = np.asarray(o32, dtype=np.float32).reshape(NG, 2, 128, S * S, 128)
    arr = v.transpose(3, 1, 2, 0, 4).reshape(S * S, C, NPIX)  # s, c, n
    a2 = arr.reshape(S * S, C // 4, 2, 2, H, W)  # s, c4, hb, wb, h, w
    return np.ascontiguousarray(
        a2.transpose(0, 1, 4, 2, 3, 5)).reshape(C, S * H, S * W)


def kernel(x, w1, b1, gamma, beta, w2, b2, **kwargs):
    in_maps = _make_in_maps(dict(x=x, w1=w1, b1=b1, gamma=gamma, beta=beta,
                                 w2=w2, b2=b2))
    nc = _get_nc()
    res = run_bass_kernel_spmd(nc, in_maps, core_ids=list(range(NCORES)))
    return np.stack([_unscramble(res.results[b]["o32"])
                     for b in range(NCORES)], axis=0)
